# revision 7
# baseline (speedup 1.0000x reference)
"""ClusterNet (vq_codebook) Trainium2 kernel — two collective-free launches.

Computes, for z (8192, 256) and centroids (64, 256):
  sim  = euclidean_dist(z, centroids)                  (8192, 64)
  Q    = rownorm(1 / (1 + sim))
  P    = rownorm(Q^2 / colsum(Q))
and returns (Q, P), matching the reference nn_ClusterNet module.

Distribution: data-parallel over the batch across 8 NeuronCores (1024
rows/core), centroids replicated.  The global column-sum of Q (64 floats
per core) is reduced on the host between two launches — the on-device
AllReduce costs ~95us/exec (NRT cc-op rendezvous; measured), and shared
DRAM is only visible within core pairs, so a second launch (~15us fixed
cost) is the cheapest global reduction.

Launch A (per core): per 128-row tile, dist^2 = (-2 z.c^T + |c|^2) is
accumulated in PSUM from bf16 matmuls (one region-wide rank-1 for the
|c|^2 term per 2-tile chunk), |z|^2 stays f32 and enters as the
per-partition bias of the ACT sqrt:  sim = Sqrt(psum + zn2).  All ACT
functions used (Copy casts / Square accum / Sqrt) live in one table set.
The z chunks are DMA'd on four different engine queues so their issue
overlaps.  U = 1/(1+sim) via DVE fast reciprocal, row-normalize to Q,
and a single f32 ones-matmul per half gives per-tile column sums
(8 x 64 row, host folds).  Outputs Q-shard + cs partials.

Launch B (per core): host sends ssb = sqrt(1/colsum); one f32 PE matmul
broadcasts it to 128 partitions; then v = Q*ssb, P = rownorm(v^2) on DVE.
"""

import os
import sys

if "/opt/trn_rl_repo" not in sys.path:
    sys.path.insert(0, "/opt/trn_rl_repo")

import numpy as np

import concourse.bass as bass
import concourse.bacc as bacc
import concourse.tile as tile
from concourse import mybir
from concourse.masks import make_identity

NCORES = 8
BS = 1024          # rows per core
T = 8              # 128-row tiles per core
TG = 2             # tiles per DMA/transpose chunk
NG = T // TG       # chunks
HT = T // 2        # tiles per half (elementwise-chain granularity)
H = 256            # feature dim
K = 64             # clusters
F32 = mybir.dt.float32
BF16 = mybir.dt.bfloat16
AF = mybir.ActivationFunctionType


def build_kernel_a():
    nc = bacc.Bacc("TRN2", target_bir_lowering=False, debug=False,
                   num_devices=NCORES)
    z_d = nc.dram_tensor("z", [BS, H], F32, kind="ExternalInput")
    c_d = nc.dram_tensor("centroids", [K, H], F32, kind="ExternalInput")
    q_d = nc.dram_tensor("qout", [BS, K], F32, kind="ExternalOutput")
    cs_d = nc.dram_tensor("cs", [T * K], F32, kind="ExternalOutput")

    with tile.TileContext(nc) as tc:
        with (
            tc.tile_pool(name="consts", bufs=1) as consts,
            tc.tile_pool(name="sb", bufs=1) as sb,
            tc.tile_pool(name="ptz", bufs=2, space="PSUM") as ptz,
            tc.tile_pool(name="psum", bufs=1, space="PSUM") as psum,
        ):
            # ---- input DMAs: z chunks on 4 queues + centroids, all issued
            # immediately so the transfers overlap ----
            z_nat = sb.tile([128, T, H], F32)
            z_t = z_d[:].rearrange("(t p) h -> t p h", p=128)
            c_nat = sb.tile([K, H], F32)
            nc.gpsimd.dma_start(out=c_nat, in_=c_d[:])
            dma_eng = [nc.sync, nc.scalar, nc.gpsimd, nc.sync]
            for g in range(NG):
                t0 = g * TG
                dma_eng[g].dma_start(
                    out=z_nat[:, t0 : t0 + TG, :],
                    in_=z_t[t0 : t0 + TG].rearrange("t p h -> p t h"),
                )

            ones_row_bf = consts.tile([1, 128], BF16)
            nc.vector.memset(ones_row_bf, 1.0)
            ones_col_f = consts.tile([128, 1], F32)
            nc.vector.memset(ones_col_f, 1.0)
            ident_bf = consts.tile([128, 128], BF16)
            make_identity(nc, ident_bf)

            # ---- centroids: cn2 row + (-2 c)^T in bf16 ----
            c_bf = sb.tile([K, H], BF16)
            nc.vector.tensor_copy(c_bf, c_nat)
            c2_bf = sb.tile([K, H], BF16)
            nc.vector.tensor_scalar_mul(c2_bf, c_bf, -2.0)
            c_sq = sb.tile([K, H], BF16)
            cn2col = sb.tile([K, 1], F32)
            nc.scalar.activation(c_sq, c_nat, AF.Square, accum_out=cn2col)
            cn2col_bf = sb.tile([K, 1], BF16)
            nc.vector.tensor_copy(cn2col_bf, cn2col)

            misc = psum.tile([128, 4, K], BF16)
            nc.tensor.transpose(misc[0:1, 2, :], cn2col_bf,
                                ident_bf[0:K, 0:K])
            cn2row_bf = sb.tile([1, K], BF16)
            nc.vector.tensor_copy(cn2row_bf, misc[0:1, 2, :])

            for j in range(2):
                nc.tensor.transpose(
                    misc[:, j, :], c2_bf[:, j * 128 : (j + 1) * 128],
                    ident_bf[0:K, 0:K],
                )
            cT2 = sb.tile([128, 2, K], BF16)
            nc.vector.tensor_copy(cT2, misc[:, 0:2, :])

            # ---- per chunk: cast (ACT), zn2 (ACT Square+accum),
            # transpose (PE), copyback (DVE), dist matmuls (PE) ----
            z_bf = sb.tile([128, T, H], BF16)
            z2scr = sb.tile([128, H], BF16)   # dead-store Square output
            zn2 = sb.tile([128, T], F32)
            zT = sb.tile([128, T, 2, 128], BF16)
            pd = [psum.tile([128, TG, K], F32, name=f"pd{g}")
                  for g in range(NG)]

            def emit_zn2(g):
                for tt in range(TG):
                    t = g * TG + tt
                    nc.scalar.activation(z2scr, z_nat[:, t, :], AF.Square,
                                         accum_out=zn2[:, t : t + 1])

            for g in range(NG):
                t0 = g * TG
                nc.scalar.copy(z_bf[:, t0 : t0 + TG, :],
                               z_nat[:, t0 : t0 + TG, :])
                if g >= 1:
                    emit_zn2(g - 1)  # keep the next cast ahead on ACT
                pzt = ptz.tile([128, 2 * TG, 128], BF16, tag="zt")
                for tt in range(TG):
                    t = t0 + tt
                    for j in range(2):
                        nc.tensor.transpose(
                            pzt[:, 2 * tt + j, :],
                            z_bf[:, t, j * 128 : (j + 1) * 128],
                            ident_bf,
                        )
                nc.vector.tensor_copy(zT[:, t0 : t0 + TG, :, :], pzt)
                # dist^2 partial: rank-1 |c|^2 over the whole chunk, then
                # per-tile -2 z.c matmuls accumulated on top
                nc.tensor.matmul(
                    pd[g][:, :, :],
                    ones_row_bf[0:1, :],
                    cn2row_bf[:, None, :].to_broadcast((1, TG, K)),
                    start=True, stop=False,
                )
                for tt in range(TG):
                    t = t0 + tt
                    nc.tensor.matmul(pd[g][:, tt, :], zT[:, t, 0, :],
                                     cT2[:, 0, :], start=False, stop=False)
                    nc.tensor.matmul(pd[g][:, tt, :], zT[:, t, 1, :],
                                     cT2[:, 1, :], start=False, stop=True)

            emit_zn2(NG - 1)

            # ---- per tile: sim = Sqrt(pd + zn2) on ACT ----
            sim = sb.tile([128, T, K], F32)
            for g in range(NG):
                for tt in range(TG):
                    t = g * TG + tt
                    nc.scalar.activation(sim[:, t, :], pd[g][:, tt, :],
                                         AF.Sqrt, bias=zn2[:, t : t + 1])

            # ---- per half: U = 1/(1+sim), Q = rownorm(U), colsum, out ----
            u1 = sb.tile([128, T * K], F32)
            u = sb.tile([128, T * K], F32)
            rU = sb.tile([128, T], F32)
            rUi = sb.tile([128, T], F32)
            q_sb = sb.tile([128, T, K], F32)
            csP = psum.tile([1, T * K], F32)
            q_out = q_d[:].rearrange("(t p) k -> p t k", p=128)
            for hh in range(2):
                sl = slice(hh * HT, (hh + 1) * HT)
                fs = slice(hh * HT * K, (hh + 1) * HT * K)
                nc.vector.tensor_scalar_add(
                    u1[:, fs],
                    sim[:, sl, :].rearrange("p t k -> p (t k)"), 1.0)
                nc.vector.reciprocal_approx_fast(out=u[:, fs], in_=u1[:, fs])
                nc.vector.reduce_sum(
                    rU[:, sl],
                    u[:, fs].rearrange("p (t k) -> p t k", k=K),
                    axis=mybir.AxisListType.X)
                nc.vector.reciprocal(rUi[:, sl], rU[:, sl])
                nc.vector.tensor_tensor(
                    out=q_sb[:, sl, :],
                    in0=u[:, fs].rearrange("p (t k) -> p t k", k=K),
                    in1=rUi[:, sl, None].to_broadcast((128, HT, K)),
                    op=mybir.AluOpType.mult,
                )
                nc.sync.dma_start(out=q_out[:, sl, :], in_=q_sb[:, sl, :])
                # per-tile column sums: ones^T @ Q (f32), host folds tiles
                nc.tensor.matmul(csP[0:1, fs], ones_col_f, q_sb[:, sl, :],
                                 start=True, stop=True)
            cs_sb = sb.tile([1, T * K], F32)
            nc.vector.tensor_copy(cs_sb, csP[0:1, :])
            nc.sync.dma_start(out=cs_d[:], in_=cs_sb)

    nc.compile()
    return nc


def build_kernel_b():
    nc = bacc.Bacc("TRN2", target_bir_lowering=False, debug=False,
                   num_devices=NCORES)
    q_d = nc.dram_tensor("q", [BS, K], F32, kind="ExternalInput")
    ssb_d = nc.dram_tensor("ssb", [K], F32, kind="ExternalInput")
    p_d = nc.dram_tensor("pout", [BS, K], F32, kind="ExternalOutput")

    with tile.TileContext(nc) as tc:
        with (
            tc.tile_pool(name="consts", bufs=1) as consts,
            tc.tile_pool(name="sb", bufs=1) as sb,
            tc.tile_pool(name="psum", bufs=1, space="PSUM") as psum,
        ):
            q_sb = sb.tile([128, T, K], F32)
            q_t = q_d[:].rearrange("(t p) k -> p t k", p=128)
            nc.sync.dma_start(out=q_sb[:, 0:HT, :], in_=q_t[:, 0:HT, :])
            nc.scalar.dma_start(out=q_sb[:, HT:T, :], in_=q_t[:, HT:T, :])
            ss_sb = sb.tile([1, K], F32)
            nc.gpsimd.dma_start(
                out=ss_sb,
                in_=bass.AP(tensor=ssb_d[:].tensor, offset=0,
                            ap=[[0, 1], [1, K]]),
            )
            ones_row_f = consts.tile([1, 128], F32)
            nc.vector.memset(ones_row_f, 1.0)
            ssP = psum.tile([128, K], F32)
            nc.tensor.matmul(ssP, ones_row_f, ss_sb, start=True, stop=True)

            v = sb.tile([128, T, K], F32)
            v2 = sb.tile([128, T, K], F32)
            rP = sb.tile([128, T], F32)
            rPi = sb.tile([128, T], F32)
            p_sb = sb.tile([128, T, K], F32)
            p_t = p_d[:].rearrange("(t p) k -> p t k", p=128)
            for hh in range(2):
                sl = slice(hh * HT, (hh + 1) * HT)
                nc.vector.tensor_tensor(
                    out=v[:, sl, :], in0=q_sb[:, sl, :],
                    in1=ssP[:, None, :].to_broadcast((128, HT, K)),
                    op=mybir.AluOpType.mult)
                nc.vector.tensor_tensor(out=v2[:, sl, :], in0=v[:, sl, :],
                                        in1=v[:, sl, :],
                                        op=mybir.AluOpType.mult)
                nc.vector.reduce_sum(rP[:, sl], v2[:, sl, :],
                                     axis=mybir.AxisListType.X)
                nc.vector.reciprocal(rPi[:, sl], rP[:, sl])
                nc.vector.tensor_tensor(
                    out=p_sb[:, sl, :], in0=v2[:, sl, :],
                    in1=rPi[:, sl, None].to_broadcast((128, HT, K)),
                    op=mybir.AluOpType.mult)
                nc.sync.dma_start(out=p_t[:, sl, :], in_=p_sb[:, sl, :])

    nc.compile()
    return nc


_NC_CACHE = {}


def _get_nc(which):
    if which not in _NC_CACHE:
        _NC_CACHE[which] = (build_kernel_a if which == "a" else build_kernel_b)()
    return _NC_CACHE[which]


def kernel(z: np.ndarray, centroids: np.ndarray):
    from concourse.bass_utils import run_bass_kernel_spmd

    z = np.ascontiguousarray(np.asarray(z, dtype=np.float32))
    centroids = np.ascontiguousarray(np.asarray(centroids, dtype=np.float32))
    assert z.shape == (NCORES * BS, H) and centroids.shape == (K, H)

    nc_a = _get_nc("a")
    in_a = [{"z": z[c * BS : (c + 1) * BS], "centroids": centroids}
            for c in range(NCORES)]
    res_a = run_bass_kernel_spmd(nc_a, in_a, core_ids=list(range(NCORES)))
    Q = np.concatenate([res_a.results[c]["qout"] for c in range(NCORES)], 0)
    s = np.sum([res_a.results[c]["cs"].reshape(T, K) for c in range(NCORES)],
               axis=(0, 1))
    ssb = np.sqrt(1.0 / s).astype(np.float32)

    nc_b = _get_nc("b")
    in_b = [{"q": np.ascontiguousarray(Q[c * BS : (c + 1) * BS]), "ssb": ssb}
            for c in range(NCORES)]
    res_b = run_bass_kernel_spmd(nc_b, in_b, core_ids=list(range(NCORES)))
    P = np.concatenate([res_b.results[c]["pout"] for c in range(NCORES)], 0)
    return (Q, P)


# revision 11
# speedup vs baseline: 1.0080x; 1.0080x over previous
"""ClusterNet (vq_codebook) Trainium2 kernel — two collective-free launches.

Computes, for z (8192, 256) and centroids (64, 256):
  sim  = euclidean_dist(z, centroids)                  (8192, 64)
  Q    = rownorm(1 / (1 + sim))
  P    = rownorm(Q^2 / colsum(Q))
and returns (Q, P), matching the reference nn_ClusterNet module.

Distribution: data-parallel over the batch across 8 NeuronCores (1024
rows/core), centroids replicated.  The global column-sum of Q is reduced
on the host between two launches — the on-device AllReduce costs
~95us/exec (NRT cc-op rendezvous; measured) and shared DRAM is only
visible within core pairs, so a second launch (~15us fixed cost) is the
cheapest global reduction.

Launch A (per core): z chunks are DMA'd on parallel queues; bf16 casts
on DVE; all transposes are single-instruction XBAR DMA transposes
(dma_start_transpose) instead of PE transpose + PSUM copyback chains.
dist^2 = (-2 z.c^T + |c|^2) accumulates in PSUM (one region-wide rank-1
per chunk for |c|^2, whose row vector comes straight from a ones-matmul
over cT^2); |z|^2 is computed on the otherwise-idle GPSIMD engine and
enters as the per-partition bias of the per-tile ACT sqrt.  A dummy
sqrt hoists the ACT table load before the data arrives.  U = 1/(1+sim)
via DVE fast reciprocal, row-normalize to Q, f32 ones-matmuls give
per-tile column sums (host folds).

Launch B (per core): host sends ssb = sqrt(1/colsum); one f32 PE matmul
broadcasts it to 128 partitions; then v = Q*ssb, P = rownorm(v^2) on DVE.
"""

import os
import sys

if "/opt/trn_rl_repo" not in sys.path:
    sys.path.insert(0, "/opt/trn_rl_repo")

import numpy as np

import concourse.bass as bass
import concourse.bacc as bacc
import concourse.tile as tile
from concourse import mybir

NCORES = 8
BS = 1024          # rows per core
T = 8              # 128-row tiles per core
TG = 2             # tiles per DMA/transpose chunk
NG = T // TG       # chunks
HT = T // 2        # tiles per half (elementwise-chain granularity)
H = 256            # feature dim
K = 64             # clusters
F32 = mybir.dt.float32
BF16 = mybir.dt.bfloat16
AF = mybir.ActivationFunctionType


def build_kernel_a():
    nc = bacc.Bacc("TRN2", target_bir_lowering=False, debug=False,
                   num_devices=NCORES)
    z_d = nc.dram_tensor("z", [BS, H], F32, kind="ExternalInput")
    c_d = nc.dram_tensor("centroids", [K, H], F32, kind="ExternalInput")
    q_d = nc.dram_tensor("qout", [BS, K], F32, kind="ExternalOutput")
    cs_d = nc.dram_tensor("cs", [T * K], F32, kind="ExternalOutput")

    with tile.TileContext(nc) as tc:
        with (
            tc.tile_pool(name="consts", bufs=1) as consts,
            tc.tile_pool(name="sb", bufs=1) as sb,
            tc.tile_pool(name="psum", bufs=1, space="PSUM") as psum,
        ):
            ones_row_bf = consts.tile([1, 128], BF16)
            nc.vector.memset(ones_row_bf, 1.0)
            ones_col_bf = consts.tile([128, 1], BF16)
            nc.vector.memset(ones_col_bf, 1.0)
            ones_col_f = consts.tile([128, 1], F32)
            nc.vector.memset(ones_col_f, 1.0)

            # hoist the sqrt ACT table load before any data dependency
            dummy = sb.tile([1, 1], F32)
            nc.scalar.activation(dummy, ones_col_f[0:1, 0:1], AF.Sqrt)

            # ---- input DMAs: z chunks + centroids on parallel queues ----
            z_nat = sb.tile([128, T, H], F32)
            z_t = z_d[:].rearrange("(t p) h -> t p h", p=128)
            c_nat = sb.tile([K, H], F32)
            nc.scalar.dma_start(out=c_nat, in_=c_d[:])
            dma_eng = [nc.sync, nc.scalar, nc.gpsimd, nc.sync]
            for g in range(NG):
                t0 = g * TG
                dma_eng[g].dma_start(
                    out=z_nat[:, t0 : t0 + TG, :],
                    in_=z_t[t0 : t0 + TG].rearrange("t p h -> p t h"),
                )

            # ---- centroids: (-2 c)^T via XBAR transpose; cn2 row via
            # ones-matmul over (cT2)^2/4 ----
            c_bf = sb.tile([K, H], BF16)
            nc.vector.tensor_copy(c_bf, c_nat)
            c2_bf = sb.tile([K, H], BF16)
            nc.vector.tensor_scalar_mul(c2_bf, c_bf, -2.0)
            cT2 = sb.tile([128, 2, K], BF16)
            nc.scalar.dma_start_transpose(out=cT2, in_=c2_bf[:])
            cT2sq = sb.tile([128, 2, K], BF16)
            nc.vector.tensor_tensor(out=cT2sq, in0=cT2, in1=cT2,
                                    op=mybir.AluOpType.mult)
            pcn = psum.tile([1, K], F32)
            for j in range(2):
                nc.tensor.matmul(pcn, ones_col_bf, cT2sq[:, j, :],
                                 start=(j == 0), stop=(j == 1))
            cn2row_bf = sb.tile([1, K], BF16)
            nc.scalar.activation(cn2row_bf, pcn, AF.Copy, bias=0.0,
                                 scale=0.25)

            # ---- per chunk: cast (DVE), zT via XBAR transpose,
            # zn2 (DVE bf16 TTR), dist matmuls (PE) ----
            z_bf = sb.tile([128, T, H], BF16)
            z2 = sb.tile([128, T, H], BF16)
            zn2 = sb.tile([128, T], F32)
            zT = sb.tile([128, T, 2, 128], BF16)
            pd = [psum.tile([128, TG, K], F32, name=f"pd{g}")
                  for g in range(NG)]
            for g in range(NG):
                t0 = g * TG
                nc.vector.tensor_copy(z_bf[:, t0 : t0 + TG, :],
                                      z_nat[:, t0 : t0 + TG, :])
                nc.sync.dma_start_transpose(
                    out=zT[:, t0 : t0 + TG, :, :],
                    in_=z_bf[:, t0 : t0 + TG, :],
                )
                nc.gpsimd.tensor_tensor(
                    out=z2[:, t0 : t0 + TG, :], in0=z_bf[:, t0 : t0 + TG, :],
                    in1=z_bf[:, t0 : t0 + TG, :], op=mybir.AluOpType.mult)
                nc.vector.reduce_sum(zn2[:, t0 : t0 + TG],
                                     z2[:, t0 : t0 + TG, :],
                                     axis=mybir.AxisListType.X)
                # dist^2 partial: rank-1 |c|^2 over the chunk, then
                # per-tile -2 z.c matmuls accumulated on top
                nc.tensor.matmul(
                    pd[g][:, :, :],
                    ones_row_bf[0:1, :],
                    cn2row_bf[:, None, :].to_broadcast((1, TG, K)),
                    start=True, stop=False,
                )
                for tt in range(TG):
                    t = t0 + tt
                    nc.tensor.matmul(pd[g][:, tt, :], zT[:, t, 0, :],
                                     cT2[:, 0, :], start=False, stop=False)
                    nc.tensor.matmul(pd[g][:, tt, :], zT[:, t, 1, :],
                                     cT2[:, 1, :], start=False, stop=True)

            # ---- per chunk: sim = Sqrt(pd + zn2), u1 = 1 + sim (ACT),
            # then U = 1/u1, Q = rownorm(U) (DVE), colsum (PE f32) ----
            sim = sb.tile([128, T, K], F32)
            u1 = sb.tile([128, T, K], F32)
            u = sb.tile([128, T, K], F32)
            rU = sb.tile([128, T], F32)
            rUi = sb.tile([128, T], F32)
            q_sb = sb.tile([128, T, K], F32)
            csP = psum.tile([1, T * K], F32)
            q_out = q_d[:].rearrange("(t p) k -> p t k", p=128)
            for g in range(NG):
                t0 = g * TG
                sl = slice(t0, t0 + TG)
                for tt in range(TG):
                    t = t0 + tt
                    nc.scalar.activation(sim[:, t, :], pd[g][:, tt, :],
                                         AF.Sqrt, bias=zn2[:, t : t + 1])
                nc.scalar.activation(u1[:, sl, :], sim[:, sl, :],
                                     AF.Identity, bias=1.0)
                nc.vector.reciprocal_approx_fast(
                    out=u[:, sl, :].rearrange("p t k -> p (t k)"),
                    in_=u1[:, sl, :].rearrange("p t k -> p (t k)"))
                nc.vector.reduce_sum(rU[:, sl], u[:, sl, :],
                                     axis=mybir.AxisListType.X)
                nc.vector.reciprocal(rUi[:, sl], rU[:, sl])
                nc.vector.tensor_tensor(
                    out=q_sb[:, sl, :],
                    in0=u[:, sl, :],
                    in1=rUi[:, sl, None].to_broadcast((128, TG, K)),
                    op=mybir.AluOpType.mult,
                )
                if g % 2 == 1:
                    hs = slice((g - 1) * TG, (g + 1) * TG)
                    hf = slice((g - 1) * TG * K, (g + 1) * TG * K)
                    nc.sync.dma_start(out=q_out[:, hs, :],
                                      in_=q_sb[:, hs, :])
                    # per-tile column sums: ones^T @ Q (f32), host folds
                    nc.tensor.matmul(csP[0:1, hf], ones_col_f,
                                     q_sb[:, hs, :], start=True, stop=True)
            cs_sb = sb.tile([1, T * K], F32)
            nc.scalar.copy(cs_sb, csP[0:1, :])
            nc.sync.dma_start(out=cs_d[:], in_=cs_sb)

    nc.compile()
    return nc


def build_kernel_b():
    nc = bacc.Bacc("TRN2", target_bir_lowering=False, debug=False,
                   num_devices=NCORES)
    q_d = nc.dram_tensor("q", [BS, K], F32, kind="ExternalInput")
    ssb_d = nc.dram_tensor("ssb", [K], F32, kind="ExternalInput")
    p_d = nc.dram_tensor("pout", [BS, K], F32, kind="ExternalOutput")

    with tile.TileContext(nc) as tc:
        with (
            tc.tile_pool(name="consts", bufs=1) as consts,
            tc.tile_pool(name="sb", bufs=1) as sb,
            tc.tile_pool(name="psum", bufs=1, space="PSUM") as psum,
        ):
            q_sb = sb.tile([128, T, K], F32)
            q_t = q_d[:].rearrange("(t p) k -> p t k", p=128)
            nc.sync.dma_start(out=q_sb[:, 0:HT, :], in_=q_t[:, 0:HT, :])
            nc.scalar.dma_start(out=q_sb[:, HT:T, :], in_=q_t[:, HT:T, :])
            ss_sb = sb.tile([1, K], F32)
            nc.gpsimd.dma_start(
                out=ss_sb,
                in_=bass.AP(tensor=ssb_d[:].tensor, offset=0,
                            ap=[[0, 1], [1, K]]),
            )
            ones_row_f = consts.tile([1, 128], F32)
            nc.vector.memset(ones_row_f, 1.0)
            ssP = psum.tile([128, K], F32)
            nc.tensor.matmul(ssP, ones_row_f, ss_sb, start=True, stop=True)

            v = sb.tile([128, T, K], F32)
            v2 = sb.tile([128, T, K], F32)
            rP = sb.tile([128, T], F32)
            rPi = sb.tile([128, T], F32)
            p_sb = sb.tile([128, T, K], F32)
            p_t = p_d[:].rearrange("(t p) k -> p t k", p=128)
            for hh in range(2):
                sl = slice(hh * HT, (hh + 1) * HT)
                nc.vector.tensor_tensor(
                    out=v[:, sl, :], in0=q_sb[:, sl, :],
                    in1=ssP[:, None, :].to_broadcast((128, HT, K)),
                    op=mybir.AluOpType.mult)
                nc.vector.tensor_tensor(out=v2[:, sl, :], in0=v[:, sl, :],
                                        in1=v[:, sl, :],
                                        op=mybir.AluOpType.mult)
                nc.vector.reduce_sum(rP[:, sl], v2[:, sl, :],
                                     axis=mybir.AxisListType.X)
                nc.vector.reciprocal(rPi[:, sl], rP[:, sl])
                nc.vector.tensor_tensor(
                    out=p_sb[:, sl, :], in0=v2[:, sl, :],
                    in1=rPi[:, sl, None].to_broadcast((128, HT, K)),
                    op=mybir.AluOpType.mult)
                nc.sync.dma_start(out=p_t[:, sl, :], in_=p_sb[:, sl, :])

    nc.compile()
    return nc


_NC_CACHE = {}


def _get_nc(which):
    if which not in _NC_CACHE:
        _NC_CACHE[which] = (build_kernel_a if which == "a" else build_kernel_b)()
    return _NC_CACHE[which]


def kernel(z: np.ndarray, centroids: np.ndarray):
    from concourse.bass_utils import run_bass_kernel_spmd

    z = np.ascontiguousarray(np.asarray(z, dtype=np.float32))
    centroids = np.ascontiguousarray(np.asarray(centroids, dtype=np.float32))
    assert z.shape == (NCORES * BS, H) and centroids.shape == (K, H)

    nc_a = _get_nc("a")
    in_a = [{"z": z[c * BS : (c + 1) * BS], "centroids": centroids}
            for c in range(NCORES)]
    res_a = run_bass_kernel_spmd(nc_a, in_a, core_ids=list(range(NCORES)))
    Q = np.concatenate([res_a.results[c]["qout"] for c in range(NCORES)], 0)
    s = np.sum([res_a.results[c]["cs"].reshape(T, K) for c in range(NCORES)],
               axis=(0, 1))
    ssb = np.sqrt(1.0 / s).astype(np.float32)

    nc_b = _get_nc("b")
    in_b = [{"q": np.ascontiguousarray(Q[c * BS : (c + 1) * BS]), "ssb": ssb}
            for c in range(NCORES)]
    res_b = run_bass_kernel_spmd(nc_b, in_b, core_ids=list(range(NCORES)))
    P = np.concatenate([res_b.results[c]["pout"] for c in range(NCORES)], 0)
    return (Q, P)
